# revision 10
# baseline (speedup 1.0000x reference)
"""Trainium2 Bass kernel: GNN mean-aggregation layer, data-parallel over 8 NeuronCores.

Computes out = relu((features + mean(embedding_look_up, axis=1)) @ kernel + bias)
for features [50000, 256], embedding_look_up [50000, 16, 256] (f32).

Sharding: node dimension split 8 x 6250; kernel/bias replicated; no collectives.

Host-side, features are pre-scaled by 16 and kernel by 1/16 so the on-chip
pipeline computes relu((16*features + sum(emb)) @ (kernel/16) + bias) — the
same result with the neighbor mean's 1/16 folded away. kernel/bias/identity
are pre-cast to bf16 on host so their loads use HWDGE (no SWDGE cast DMA).

The kernel is HBM-bandwidth-bound (per-core traffic ~111 MB at ~358 GB/s).
Tiles are 126 nodes (not 128): SDMA engine 15 runs ~17% slower than engines
0-14 (SWDGE descriptor-ring port contention). Engine 15 serves SBUF
partitions {92-95, 124-127}; a 126-partition tile gives it 6 rows per tile
instead of 8, rebalancing the per-engine finish times.

Per-core pipeline, tiled over 126-node blocks (50 tiles, last one overlaps
its predecessor so all tiles are full):
  - one SWDGE DMA loads the [126, 16*256] neighbor slab, casting f32 -> bf16
    in the DMA datapath (halves SBUF write traffic),
  - VectorE reduces the 16 neighbor groups with a bf16 binary add tree (2x
    perf mode) and adds the pre-scaled self features -> X [126, 256] bf16,
  - TensorE transposes X (two 126x128 bf16 identity matmuls), ScalarE
    evacuates X^T to SBUF,
  - TensorE computes X @ W in bf16 (two K=128 single-pass matmuls) and adds
    bias with a rank-1 bf16 matmul into the same PSUM bank,
  - ScalarE applies relu (f32 out), DMA stores the [126, 256] tile.
Features load / result store are batched GROUP=7 tiles per HWDGE DMA.
"""

import numpy as np

import concourse.bacc as bacc
import concourse.mybir as mybir
from concourse import tile
from concourse.bass_utils import run_bass_kernel_spmd

N_CORES = 8
N_NODES = 50000
PER_CORE = N_NODES // N_CORES  # 6250
MAX_NEIGH = 16
D = 256
# 126 nodes per tile, not 128: SDMA engine k serves SBUF partitions p==k
# (mod 16), and engine 15 runs ~10-17% slower than the rest (SWDGE
# descriptor-ring port contention). Height 126 drops one partition row from
# engines 14 and 15, rebalancing per-engine finish times. (127 would be
# ideal — only engine 15 trimmed — but odd partition counts crash a walrus
# backend pass.)
P = 126
F32 = mybir.dt.float32
BF16 = mybir.dt.bfloat16


GROUP = 7  # tiles per batched feat-load / result-store DMA


def _tile_groups():
    """(group_offset, n_tiles, tile_height) per group. Full groups cover GROUP
    consecutive P-node tiles (batched ~0.9 MB feat/out DMAs); the ragged tail
    is a single partial tile (no re-read of rows already covered).

    Within a group, partition p of tile j handles node g0 + p*L + j (p-major)
    so each partition's L feat/result rows are contiguous in DRAM — the
    feat-load and result-store DMAs then use L*1KB descriptors, not 1KB."""
    offs = list(range(0, PER_CORE - P + 1, P))
    rem = PER_CORE - (offs[-1] + P)
    groups = [
        (offs[i], len(offs[i : i + GROUP]), P) for i in range(0, len(offs), GROUP)
    ]
    if rem:
        groups.append((offs[-1] + P, 1, rem))
    return groups


def build_nc():
    nc = bacc.Bacc(None, target_bir_lowering=False)

    feat_d = nc.declare_dram_parameter("features", [PER_CORE, D], F32, isOutput=False)
    emb_d = nc.declare_dram_parameter(
        "embedding_look_up", [PER_CORE, MAX_NEIGH, D], F32, isOutput=False
    )
    w_d = nc.declare_dram_parameter("kernel", [2, 128, D], BF16, isOutput=False)
    bias_d = nc.declare_dram_parameter("bias", [D], BF16, isOutput=False)
    id_d = nc.declare_dram_parameter("ident", [P, P], BF16, isOutput=False)
    out_d = nc.declare_dram_parameter("out", [PER_CORE, D], F32, isOutput=True)

    with tile.TileContext(nc) as tc:
        with (
            tc.tile_pool(name="const", bufs=1) as const_pool,
            tc.tile_pool(name="acc", bufs=5) as acc_pool,
            tc.tile_pool(name="feat", bufs=3) as feat_pool,
            tc.tile_pool(name="featb", bufs=3) as featb_pool,
            tc.tile_pool(name="tree", bufs=3) as tree_pool,
            tc.tile_pool(name="x", bufs=3) as x_pool,
            tc.tile_pool(name="xt", bufs=3) as xt_pool,
            tc.tile_pool(name="res", bufs=3) as res_pool,
            tc.tile_pool(name="ps_t", bufs=2, space="PSUM") as ps_t_pool,
            tc.tile_pool(name="ps_o", bufs=2, space="PSUM") as ps_o_pool,
        ):
            # Constants, all bf16 host-side -> plain HWDGE loads.
            w_sb = const_pool.tile([128, 2, D], BF16)  # w_sb[k, b, o] = W[128b + k, o]
            nc.sync.dma_start(out=w_sb, in_=w_d.rearrange("b k o -> k b o"))
            bias_sb = const_pool.tile([1, D], BF16)
            nc.sync.dma_start(out=bias_sb, in_=bias_d[None, :])
            ones_sb = const_pool.tile([1, P], BF16)
            nc.vector.memset(ones_sb, 1.0)
            id_sb = const_pool.tile([P, P], BF16)
            nc.sync.dma_start(out=id_sb, in_=id_d[:])

            # Result stores are issued one group late (after the NEXT group's
            # feature load) so a store's semaphore wait can never hold the
            # next feature load hostage on the FIFO HWDGE ring.
            pending = None  # (g0, rows, L, res_g)

            def flush_pending():
                nonlocal pending
                if pending is None:
                    return
                g0, rows, L, res_g = pending
                nc.sync.dma_start(
                    out=out_d[g0 : g0 + rows].rearrange("(p j) k -> p j k", j=L),
                    in_=res_g[: rows // L, :L, :],
                )
                pending = None

            for g0, L, h in _tile_groups():
                rows = L * h
                # Features for the whole group in one HWDGE DMA (p-major:
                # L*1KB contiguous per partition). Results accumulate in
                # res_g and leave in one batched DMA issued during the next
                # group.
                feat_g = feat_pool.tile([P, GROUP, D], F32, tag="feat_g")
                nc.sync.dma_start(
                    out=feat_g[:h, :L, :],
                    in_=feat_d[g0 : g0 + rows].rearrange("(p j) k -> p j k", j=L),
                )
                flush_pending()
                res_g = res_pool.tile([P, GROUP, D], F32, tag="res_g")
                emb_g = emb_d[g0 : g0 + rows].rearrange("(p j) m k -> p j m k", j=L)

                for j in range(L):
                    # Neighbor slab: SWDGE DMA casting f32 -> bf16 in the
                    # DMA datapath (halves SBUF write traffic). One tile per
                    # DMA — pairing slabs into 4 MB transfers measured
                    # ~40 us slower (3-dim SWDGE descriptor pattern).
                    acc = acc_pool.tile([P, MAX_NEIGH, D], BF16)
                    nc.gpsimd.dma_start(out=acc[:h], in_=emb_g[:, j])
                    featb = featb_pool.tile([P, D], BF16, tag="featb")
                    nc.scalar.copy(out=featb[:h], in_=feat_g[:h, j, :])

                    # Binary tree reduction of the 16 neighbor groups on
                    # VectorE (bf16, DVE 2x perf mode).
                    cur = acc
                    g = MAX_NEIGH
                    while g > 2:
                        nxt = tree_pool.tile([P, g // 2, D], BF16, tag=f"tree{g}")
                        nc.vector.tensor_add(
                            out=nxt[:h],
                            in0=cur[:h, 0 : g // 2, :],
                            in1=cur[:h, g // 2 : g, :],
                        )
                        cur, g = nxt, g // 2
                    t3 = tree_pool.tile([P, D], BF16, tag="t3")
                    nc.vector.tensor_add(
                        out=t3[:h], in0=cur[:h, 0, :], in1=cur[:h, 1, :]
                    )
                    # X = sum(emb) + 16*features  (features pre-scaled on host)
                    x = x_pool.tile([P, D], BF16)
                    nc.vector.tensor_add(out=x[:h], in0=t3[:h], in1=featb[:h])

                    # X^T via TensorE transpose; ScalarE evacuates to SBUF.
                    # Transpose of [h, 128] chunk -> [128, h].
                    ps_t = ps_t_pool.tile([128, 2, P], BF16)
                    for c in range(2):
                        nc.tensor.transpose(
                            ps_t[:, c, :h],
                            x[:h, 128 * c : 128 * (c + 1)],
                            id_sb[:h, :h],
                        )
                    xt = xt_pool.tile([128, 2, P], BF16)
                    nc.scalar.copy(out=xt[:, :, :h], in_=ps_t[:, :, :h])

                    # res_g[:, j] = X @ W' + bias in bf16 (f32 PSUM accumulate).
                    ps_o = ps_o_pool.tile([P, D], F32)
                    for c in range(2):
                        nc.tensor.matmul(
                            ps_o[:h],
                            xt[:, c, :h],
                            w_sb[:, c, :],
                            start=(c == 0),
                            stop=False,
                        )
                    nc.tensor.matmul(
                        ps_o[:h], ones_sb[:, :h], bias_sb, start=False, stop=True
                    )

                    nc.scalar.activation(
                        out=res_g[:h, j, :],
                        in_=ps_o[:h],
                        func=mybir.ActivationFunctionType.Relu,
                    )

                pending = (g0, rows, L, res_g)
            flush_pending()

    nc.finalize()
    return nc


def _make_in_maps(features, embedding_look_up, kernel, bias):
    # Fold the neighbor-mean's 1/16 into host-side scaling: the device
    # computes (16*features + sum(emb)) @ (kernel/16) + bias.
    import ml_dtypes

    features = np.asarray(features, dtype=np.float32) * np.float32(MAX_NEIGH)
    emb = np.ascontiguousarray(np.asarray(embedding_look_up, dtype=np.float32))
    kern = (np.asarray(kernel, dtype=np.float32) / np.float32(MAX_NEIGH)).reshape(
        2, 128, D
    ).astype(ml_dtypes.bfloat16)
    bias = np.ascontiguousarray(np.asarray(bias, dtype=np.float32)).astype(
        ml_dtypes.bfloat16
    )
    ident = np.eye(P, dtype=ml_dtypes.bfloat16)
    in_maps = []
    for c in range(N_CORES):
        sl = slice(c * PER_CORE, (c + 1) * PER_CORE)
        in_maps.append(
            {
                "features": features[sl],
                "embedding_look_up": emb[sl],
                "kernel": kern,
                "bias": bias,
                "ident": ident,
            }
        )
    return in_maps


_NC_CACHE = None


def run(inputs: dict, trace: bool = False, fresh: bool = False):
    """Build, compile and run on 8 cores; returns (full_output, BassKernelResults)."""
    global _NC_CACHE
    in_maps = _make_in_maps(
        inputs["features"],
        inputs["embedding_look_up"],
        inputs["kernel"],
        inputs["bias"],
    )
    if fresh or _NC_CACHE is None:
        _NC_CACHE = build_nc()
    res = run_bass_kernel_spmd(
        _NC_CACHE, in_maps, core_ids=list(range(N_CORES)), trace=trace
    )
    out = np.concatenate([r["out"] for r in res.results], axis=0)
    return out, res


def _spot_check(out, inputs) -> bool:
    """Cheap host-side check of 64 rows; catches (rare) silent device-side
    corruption so the caller can retry. bf16 pipeline error is ~3e-3."""
    idx = np.linspace(0, N_NODES - 1, 64).astype(np.int64)
    f = np.asarray(inputs["features"], np.float32)[idx]
    e = np.asarray(inputs["embedding_look_up"], np.float32)[idx]
    w = np.asarray(inputs["kernel"], np.float32)
    b = np.asarray(inputs["bias"], np.float32)
    exp = np.maximum((f + e.mean(axis=1)) @ w + b, 0.0)
    denom = max(np.abs(exp).max(), 1e-6)
    return np.abs(out[idx] - exp).max() / denom < 3e-2


def kernel(**inputs) -> np.ndarray:
    try:
        out, _ = run(inputs)
        if _spot_check(out, inputs):
            return out
    except Exception:
        # Transient NRT/device errors usually clear on a fresh attempt.
        pass
    out, _ = run(inputs, fresh=True)
    return out
